# revision 2
# baseline (speedup 1.0000x reference)
"""Trainium2 Bass kernel for nn_ControlFlexHNN (dense_mlp).

Data-parallel across 8 NeuronCores: batch N=32768 -> 4096 rows/core.
Activations are feature-major on-chip ([feature, batch]); every matmul
contracts over the partition dimension.

v2 design (vs f32r baseline):
  - All matmul operands bf16 (same PE rate as f32r at free>=256, half
    the SBUF), PSUM accumulation fp32, elementwise chain fp32 where
    cancellation matters (s = 1 - tanh^2 computed from fp32 tanh).
  - Wh folded into W2 host-side (W2W = W2 * Wh[:,None]) so the backward
    matmul contracts s1 directly - no ga2 array or scaling op.
  - Biases of the small (contract 16/20) layers folded into the matmul
    via a ones-row in zu ([z(16), 1, u(4)] layout).
  - j-chunks processed in pairs: elementwise ops run at width 1024,
    halving instruction count + fixed overheads.
  - Stage A (layer-1 fwd) of tile t+1 rides inside stage E of tile t,
    and stage C (flex layer-1) mms sit right next to stage B's, so the
    PE never idles at stage boundaries.  Activation arrays are
    double-buffered (bufs=2) to allow the cross-tile overlap.

Device kernel per core (B=512 batch tile, 8 tiles):
  A: pa1 = W1x @ [z;1]         h1f = tanh(pa1); s0 = 1-h1f^2; h1 = bf16(h1f)
  B: pb = W2 @ h1 (+b2 bias)   s1 = 1-tanh^2(pb+b2)
  C: pf = Wf1x @ [z;1;u]       g1 = tanh(pf) + pf*s1
  D: pg = W2W.T @ s1           ga1 = pg*s0;  ps  = W1.T @ ga1   (dH)
  E: pf2 = Wf2 @ g1 (+b2f)     g2 = tanh(pf2+bf2) + (pf2+bf2)*s0
                               ps += Wff @ g2;  out = ps + bff
"""

import numpy as np

N = 32768
DQ = 8
D2 = 2 * DQ          # 16
A_DIM = 4
Z1 = D2 + 1          # 17 rows: z + ones   (layer-A lhs contract)
ZU1 = D2 + 1 + A_DIM  # 21 rows: z + ones + u (layer-C lhs contract)
H = 1024
HC = H // 128        # 8 chunks
NP = 2               # chunks per elementwise pair
PAIRS = HC // NP     # 4
NCORES = 8
NSH = N // NCORES    # 4096 rows per core
B = 512              # batch tile (free dim of matmuls)
TILES = NSH // B     # 8

_BUILT = None


def _build(loop_n=None):
    """Build the kernel. loop_n wraps the whole 8-tile body in an on-device
    For_i loop (used only for HW timing via replication differencing)."""
    import contextlib

    import concourse.bacc as bacc
    import concourse.mybir as mybir
    from concourse import tile

    f32 = mybir.dt.float32
    bf16 = mybir.dt.bfloat16

    nc = bacc.Bacc(None)

    zut_d = nc.dram_tensor("zut", [ZU1, NSH], bf16, kind="ExternalInput")
    w1t_d = nc.dram_tensor("w1t", [Z1, H], bf16, kind="ExternalInput")
    w1n_d = nc.dram_tensor("w1n", [H, D2], bf16, kind="ExternalInput")
    w2t_d = nc.dram_tensor("w2t", [H, H], bf16, kind="ExternalInput")
    w2wn_d = nc.dram_tensor("w2wn", [H, H], bf16, kind="ExternalInput")
    wf1t_d = nc.dram_tensor("wf1t", [ZU1, H], bf16, kind="ExternalInput")
    wf2t_d = nc.dram_tensor("wf2t", [H, H], bf16, kind="ExternalInput")
    wfft_d = nc.dram_tensor("wfft", [H, D2], bf16, kind="ExternalInput")
    b2c_d = nc.dram_tensor("b2c", [128, HC], f32, kind="ExternalInput")
    bf2c_d = nc.dram_tensor("bf2c", [128, HC], f32, kind="ExternalInput")
    bffc_d = nc.dram_tensor("bffc", [D2, 1], f32, kind="ExternalInput")
    st_d = nc.dram_tensor("st", [D2, NSH], f32, kind="ExternalOutput")

    with tile.TileContext(nc) as tc:
        with (
            tc.tile_pool(name="wp", bufs=1) as wp,
            tc.tile_pool(name="actp", bufs=2) as actp,
            tc.tile_pool(name="tmpp", bufs=2) as tmpp,
            tc.tile_pool(name="grp", bufs=3) as grp,
            tc.tile_pool(name="iop", bufs=2) as iop,
            tc.tile_pool(name="mmp", bufs=3, space="PSUM") as mmp,
            tc.tile_pool(name="accp", bufs=2, space="PSUM") as accp,
        ):
            # ---- resident weights ----
            w1t = wp.tile([Z1, H], bf16)
            nc.sync.dma_start(w1t[:], w1t_d[:])
            w1n = wp.tile([128, HC, D2], bf16)
            nc.sync.dma_start(w1n[:], w1n_d.rearrange("(c p) m -> p c m", p=128))
            w2t = wp.tile([128, HC, H], bf16)
            nc.sync.dma_start(w2t[:], w2t_d.rearrange("(c p) j -> p c j", p=128))
            w2wn = wp.tile([128, HC, H], bf16)
            nc.sync.dma_start(w2wn[:], w2wn_d.rearrange("(c p) k -> p c k", p=128))
            wf1t = wp.tile([ZU1, H], bf16)
            nc.sync.dma_start(wf1t[:], wf1t_d[:])
            wf2t = wp.tile([128, HC, H], bf16)
            nc.sync.dma_start(wf2t[:], wf2t_d.rearrange("(c p) j -> p c j", p=128))
            wfft = wp.tile([128, HC, D2], bf16)
            nc.sync.dma_start(wfft[:], wfft_d.rearrange("(c p) m -> p c m", p=128))
            b2c = wp.tile([128, HC], f32)
            nc.sync.dma_start(b2c[:], b2c_d[:])
            bf2c = wp.tile([128, HC], f32)
            nc.sync.dma_start(bf2c[:], bf2c_d[:])
            bffc = wp.tile([D2, 1], f32)
            nc.sync.dma_start(bffc[:], bffc_d[:])

            ws = (w1t, w1n, w2t, w2wn, wf1t, wf2t, wfft, b2c, bf2c, bffc)
            pools = (actp, tmpp, grp, iop, mmp, accp)

            loop_cm = tc.For_i(0, loop_n, 1) if loop_n else contextlib.nullcontext()
            with loop_cm:
                _emit_body(nc, mybir, pools, ws, zut_d, st_d)

    nc.compile()
    return nc


def _build_looped(loop_n):
    return _build(loop_n=loop_n)


def _emit_A(nc, mybir, actp, tmpp, mmp, w1t, zut, h1, s0, i):
    """Stage A for chunk-pair i: pa1 = W1x @ [z;1]; h1f = tanh; s0; h1."""
    f32 = mybir.dt.float32
    Tanh = mybir.ActivationFunctionType.Tanh
    Square = mybir.ActivationFunctionType.Square
    mult = mybir.AluOpType.mult
    add = mybir.AluOpType.add

    pa = mmp.tile([128, NP, B], f32, tag="mm", name=f"pa_{i}")
    for h in range(NP):
        j = NP * i + h
        nc.tensor.matmul(pa[:, h, :], w1t[:, j * 128:(j + 1) * 128],
                         zut[0:Z1, :], start=True, stop=True)
    h1f = tmpp.tile([128, NP, B], f32, tag="h1f", name=f"h1f_{i}")
    nc.scalar.activation(h1f[:], pa[:], Tanh)
    q2 = tmpp.tile([128, NP, B], f32, tag="q", name=f"q2_{i}")
    nc.scalar.activation(q2[:], h1f[:], Square)
    sl = slice(NP * i, NP * i + NP)
    nc.vector.tensor_scalar(out=s0[:, sl, :], in0=q2[:], scalar1=-1.0,
                            scalar2=1.0, op0=mult, op1=add)
    nc.gpsimd.tensor_copy(h1[:, sl, :], h1f[:])


def _emit_body(nc, mybir, pools, ws, zut_d, st_d):
    f32 = mybir.dt.float32
    bf16 = mybir.dt.bfloat16
    Tanh = mybir.ActivationFunctionType.Tanh
    Square = mybir.ActivationFunctionType.Square
    Ident = mybir.ActivationFunctionType.Identity
    mult = mybir.AluOpType.mult
    add = mybir.AluOpType.add

    actp, tmpp, grp, iop, mmp, accp = pools
    w1t, w1n, w2t, w2wn, wf1t, wf2t, wfft, b2c, bf2c, bffc = ws

    # activation arrays, double-buffered across tiles via the pool
    def arrs(t):
        h1 = actp.tile([128, HC, B], bf16, tag="h1", name=f"h1_{t}")
        s0 = actp.tile([128, HC, B], bf16, tag="s0", name=f"s0_{t}")
        return h1, s0

    # ---- prologue: zut(0) + A(0) ----
    zut = iop.tile([ZU1, B], bf16, tag="zut", name="zut_0")
    nc.sync.dma_start(zut[:], zut_d[:, 0:B])
    h1, s0 = arrs(0)
    for i in range(PAIRS):
        _emit_A(nc, mybir, actp, tmpp, mmp, w1t, zut, h1, s0, i)

    for t in range(TILES):
        sl_t = slice(t * B, (t + 1) * B)
        # prefetch zut(t+1)
        if t + 1 < TILES:
            zut_n = iop.tile([ZU1, B], bf16, tag="zut", name=f"zut_{t + 1}")
            nc.sync.dma_start(zut_n[:], zut_d[:, (t + 1) * B:(t + 2) * B])
        else:
            zut_n = None

        s1 = actp.tile([128, HC, B], bf16, tag="s1", name=f"s1_{t}")
        g1 = actp.tile([128, HC, B], bf16, tag="g1", name=f"g1_{t}")

        # ---- B+C: a2 -> s1; f1 -> g1 ----
        for i in range(PAIRS):
            pb = mmp.tile([128, NP, B], f32, tag="mm", name=f"pb_{t}_{i}")
            for h in range(NP):
                j = NP * i + h
                for k in range(HC):
                    nc.tensor.matmul(pb[:, h, :],
                                     w2t[:, k, j * 128:(j + 1) * 128],
                                     h1[:, k, :], start=(k == 0),
                                     stop=(k == HC - 1))
            pf = mmp.tile([128, NP, B], f32, tag="mm", name=f"pf_{t}_{i}")
            for h in range(NP):
                j = NP * i + h
                nc.tensor.matmul(pf[:, h, :], wf1t[:, j * 128:(j + 1) * 128],
                                 zut[:], start=True, stop=True)
            h2f = tmpp.tile([128, NP, B], f32, tag="h2f", name=f"h2f_{t}_{i}")
            for h in range(NP):
                j = NP * i + h
                nc.scalar.activation(h2f[:, h, :], pb[:, h, :], Tanh,
                                     bias=b2c[:, j:j + 1])
            qb = tmpp.tile([128, NP, B], f32, tag="q", name=f"qb_{t}_{i}")
            nc.scalar.activation(qb[:], h2f[:], Square)
            psl = slice(NP * i, NP * i + NP)
            nc.vector.tensor_scalar(out=s1[:, psl, :], in0=qb[:], scalar1=-1.0,
                                    scalar2=1.0, op0=mult, op1=add)
            th = tmpp.tile([128, NP, B], f32, tag="th", name=f"th_{t}_{i}")
            nc.scalar.activation(th[:], pf[:], Tanh)
            pm = tmpp.tile([128, NP, B], f32, tag="pm", name=f"pm_{t}_{i}")
            nc.vector.tensor_tensor(out=pm[:], in0=pf[:], in1=s1[:, psl, :],
                                    op=mult)
            nc.gpsimd.tensor_tensor(out=g1[:, psl, :], in0=th[:], in1=pm[:],
                                    op=add)

        # ---- D: pg = W2W.T @ s1; ga1 = pg*s0; ps = W1.T @ ga1 ----
        ps = accp.tile([D2, B], f32, tag="acc", name=f"ps_{t}")
        ga1_prev = None
        for i in range(PAIRS):
            pg = mmp.tile([128, NP, B], f32, tag="mm", name=f"pg_{t}_{i}")
            for h in range(NP):
                k = NP * i + h
                for j in range(HC):
                    nc.tensor.matmul(pg[:, h, :],
                                     w2wn[:, j, k * 128:(k + 1) * 128],
                                     s1[:, j, :], start=(j == 0),
                                     stop=(j == HC - 1))
            ga1 = grp.tile([128, NP, B], bf16, tag="gr", name=f"ga1_{t}_{i}")
            psl = slice(NP * i, NP * i + NP)
            nc.vector.tensor_tensor(out=ga1[:], in0=pg[:], in1=s0[:, psl, :],
                                    op=mult)
            if i >= 1:
                for h in range(NP):
                    k = NP * (i - 1) + h
                    nc.tensor.matmul(ps[:], w1n[:, k, :], ga1_prev[:, h, :],
                                     start=(i == 1 and h == 0), stop=False)
            ga1_prev = ga1
        for h in range(NP):
            k = NP * (PAIRS - 1) + h
            nc.tensor.matmul(ps[:], w1n[:, k, :], ga1_prev[:, h, :],
                             start=False, stop=False)

        # ---- E: f2 -> g2 -> head accum; stage A(t+1) rides along ----
        if t + 1 < TILES:
            h1_n, s0_n = arrs(t + 1)
        g2_prev = None
        for i in range(PAIRS):
            pf2 = mmp.tile([128, NP, B], f32, tag="mm", name=f"pf2_{t}_{i}")
            for h in range(NP):
                j = NP * i + h
                for k in range(HC):
                    nc.tensor.matmul(pf2[:, h, :],
                                     wf2t[:, k, j * 128:(j + 1) * 128],
                                     g1[:, k, :], start=(k == 0),
                                     stop=(k == HC - 1))
            if t + 1 < TILES:
                pa = mmp.tile([128, NP, B], f32, tag="mm", name=f"pa_{t + 1}_{i}")
                for h in range(NP):
                    j = NP * i + h
                    nc.tensor.matmul(pa[:, h, :],
                                     w1t[:, j * 128:(j + 1) * 128],
                                     zut_n[0:Z1, :], start=True, stop=True)
            th2 = tmpp.tile([128, NP, B], f32, tag="th", name=f"th2_{t}_{i}")
            for h in range(NP):
                j = NP * i + h
                nc.scalar.activation(th2[:, h, :], pf2[:, h, :], Tanh,
                                     bias=bf2c[:, j:j + 1])
            if t + 1 < TILES:
                h1f = tmpp.tile([128, NP, B], f32, tag="h1f",
                                name=f"h1f_{t + 1}_{i}")
                nc.scalar.activation(h1f[:], pa[:], Tanh)
                q2 = tmpp.tile([128, NP, B], f32, tag="q",
                               name=f"q2_{t + 1}_{i}")
                nc.scalar.activation(q2[:], h1f[:], Square)
            pm2 = tmpp.tile([128, NP, B], f32, tag="pm", name=f"pm2_{t}_{i}")
            psl = slice(NP * i, NP * i + NP)
            for h in range(NP):
                j = NP * i + h
                nc.vector.scalar_tensor_tensor(out=pm2[:, h, :],
                                               in0=pf2[:, h, :],
                                               scalar=bf2c[:, j:j + 1],
                                               in1=s0[:, j, :],
                                               op0=add, op1=mult)
            if t + 1 < TILES:
                nc.vector.tensor_scalar(out=s0_n[:, psl, :], in0=q2[:],
                                        scalar1=-1.0, scalar2=1.0,
                                        op0=mult, op1=add)
            g2 = grp.tile([128, NP, B], bf16, tag="gr", name=f"g2_{t}_{i}")
            nc.gpsimd.tensor_tensor(out=g2[:], in0=th2[:], in1=pm2[:], op=add)
            if t + 1 < TILES:
                nc.gpsimd.tensor_copy(h1_n[:, psl, :], h1f[:])
            if i >= 1:
                for h in range(NP):
                    j = NP * (i - 1) + h
                    nc.tensor.matmul(ps[:], wfft[:, j, :], g2_prev[:, h, :],
                                     start=False, stop=False)
            g2_prev = g2
        for h in range(NP):
            j = NP * (PAIRS - 1) + h
            nc.tensor.matmul(ps[:], wfft[:, j, :], g2_prev[:, h, :],
                             start=False, stop=(h == NP - 1))

        sout = iop.tile([D2, B], f32, tag="sout", name=f"sout_{t}")
        nc.scalar.activation(sout[:], ps[:], Ident, bias=bffc[:, 0:1])
        nc.sync.dma_start(st_d[:, sl_t], sout[:])

        # roll state for next tile
        if t + 1 < TILES:
            zut = zut_n
            h1, s0 = h1_n, s0_n


def _prep_inputs(t, z, W1, b1, W2, b2, Wh, bh, Wf1, bf1, Wf2, bf2, Wff, bff,
                 Wp, bp):
    import ml_dtypes
    f = np.float32
    bf = ml_dtypes.bfloat16
    z = np.asarray(z, f)
    u = np.tanh(z @ np.asarray(Wp, f).T + np.asarray(bp, f))
    ones = np.ones((z.shape[0], 1), f)
    zu1 = np.concatenate([z, ones, u], axis=1)   # [N, 21] rows: z, 1, u

    def cb(x):
        return np.ascontiguousarray(np.asarray(x, f).astype(bf))

    def cf(x):
        return np.ascontiguousarray(np.asarray(x, f))

    W1 = np.asarray(W1, f); b1 = np.asarray(b1, f)
    W2 = np.asarray(W2, f); b2 = np.asarray(b2, f)
    Wh = np.asarray(Wh, f)
    Wf1 = np.asarray(Wf1, f); bf1 = np.asarray(bf1, f)
    Wf2 = np.asarray(Wf2, f); bf2 = np.asarray(bf2, f)
    Wff = np.asarray(Wff, f); bff = np.asarray(bff, f)

    # layer-A lhsT rows: z-features then the bias row
    w1t = np.concatenate([W1.T, b1[None, :]], axis=0)          # [17, H]
    # layer-C lhsT rows must match zu1 row order: z(16), ones, u(4)
    wf1t = np.concatenate([Wf1.T[:D2], bf1[None, :], Wf1.T[D2:]], axis=0)
    w2w = W2 * Wh.reshape(H, 1)                                 # [H, H]

    shared = {
        "w1t": cb(w1t),
        "w1n": cb(W1),
        "w2t": cb(W2.T),
        "w2wn": cb(w2w),
        "wf1t": cb(wf1t),
        "wf2t": cb(Wf2.T),
        "wfft": cb(Wff.T),
        "b2c": cf(b2.reshape(HC, 128).T),
        "bf2c": cf(bf2.reshape(HC, 128).T),
        "bffc": cf(bff.reshape(D2, 1)),
    }
    in_maps = []
    for r in range(NCORES):
        m = dict(shared)
        m["zut"] = np.ascontiguousarray(
            zu1[r * NSH:(r + 1) * NSH].T.astype(bf))
        in_maps.append(m)
    return in_maps


def _postprocess(results):
    outs = []
    for r in range(NCORES):
        s = results[r]["st"].T                    # [NSH, 16]
        outs.append(np.concatenate([s[:, DQ:], -s[:, :DQ]], axis=1))
    return np.ascontiguousarray(np.concatenate(outs, axis=0).astype(np.float32))


def kernel(**inputs):
    global _BUILT
    from concourse.bass_utils import run_bass_kernel_spmd

    if _BUILT is None:
        _BUILT = _build()
    in_maps = _prep_inputs(**inputs)
    res = run_bass_kernel_spmd(_BUILT, in_maps, list(range(NCORES)))
    return _postprocess(res.results)
